# revision 10
# baseline (speedup 1.0000x reference)
"""Trainium2 Bass kernel for the GeneGroupModel two-layer problem.

Model: g = relu(segment_sum(x * w_flat, seg) + gene_b)
       h1 = relu(BN(g @ W1.T + b1));  h2 = relu(BN(h1 @ W2.T + b2))
       out = h2 @ Wout.T + bout            (BN uses full-batch statistics)

Strategy (8 NeuronCores, data-parallel over the batch):
 - batch B=2048 sharded 8 x 256 rows.
 - x is transposed + bf16-cast on the host into a [128, 469*256] layout
   (partition p, column c*256+b  =  x[b, 128c+p]) so the device streams
   it with large fully-contiguous DMAs (~1MB per super-chunk) at HBM
   line rate.  No DMA-transpose.
 - The segment structure repeats exactly every 120 features (sizes
   16/24/32/48) -> 1920 features == 64 groups.  The segment-sum becomes
   a band matmul: per 128-feature chunk, psg[64, 256] += Wband.T @ xT
   where Wband = diag(w_chunk) @ IND and IND is ONE constant [1920, 64]
   0/1 block shared by all super-chunks.  Wband is built on-chip.
 - The MLP layer-1 matmul is interleaved into the main loop (each
   completed 128-group tile of g immediately feeds 4 accumulating
   matmuls into 4 persistent PSUM banks), so only BN + layer-2 + head
   remain as a tail.  Weights and activations use bf16 (f32 PSUM
   accumulation); BN batch statistics are summed across cores with two
   tiny AllReduce collectives.
 - b1/b2 are omitted: BN subtracts the batch mean, so a constant bias
   added before BN cancels exactly.
"""

import numpy as np
import ml_dtypes

import concourse.bass as bass
import concourse.bacc as bacc
import concourse.mybir as mybir
from concourse import tile
from concourse.bass_utils import run_bass_kernel_spmd

F32 = mybir.dt.float32
BF16 = mybir.dt.bfloat16

B, F, G = 2048, 60000, 2000
H1, H2 = 512, 256
EPS = 1e-5
NCORES = 8
BS = B // NCORES            # 256 batch rows per core
NSUB = 469                  # ceil(F/128); F padded to FP
FP = NSUB * 128             # 60032
SUPER_SUBS = 15             # 15 x 128 = 1920 features per super-chunk
NSUPER = 32                 # 31 full + 1 tail (4 subchunks, 16 groups)
GBLK = 64                   # groups per full super-chunk
GT_TILES = 16               # partition tiles of gT (G padded to 2048)

_SIZES = np.tile(np.array([16, 24, 32, 48], np.int64), 500)


def _build_graph():
    nc = bacc.Bacc("TRN2", target_bir_lowering=False, debug=False,
                   num_devices=NCORES)
    x_d = nc.declare_dram_parameter("x", [128, NSUB * BS], BF16, isOutput=False)
    ind_d = nc.declare_dram_parameter("ind", [128, SUPER_SUBS * GBLK], F32, isOutput=False)
    wpt_d = nc.declare_dram_parameter("wpt", [128, NSUB], F32, isOutput=False)
    gbpt_d = nc.declare_dram_parameter("gbpt", [128, GT_TILES], F32, isOutput=False)
    w1t_d = nc.declare_dram_parameter("w1t", [128, GT_TILES * H1], BF16, isOutput=False)
    g1pt_d = nc.declare_dram_parameter("g1pt", [128, 4], F32, isOutput=False)
    be1pt_d = nc.declare_dram_parameter("be1pt", [128, 4], F32, isOutput=False)
    w2t_d = nc.declare_dram_parameter("w2t", [128, 4 * H2], BF16, isOutput=False)
    g2pt_d = nc.declare_dram_parameter("g2pt", [128, 2], F32, isOutput=False)
    be2pt_d = nc.declare_dram_parameter("be2pt", [128, 2], F32, isOutput=False)
    wopt_d = nc.declare_dram_parameter("wopt", [128, 2], BF16, isOutput=False)
    bout_d = nc.declare_dram_parameter("boutv", [1, 1], F32, isOutput=False)
    out_d = nc.declare_dram_parameter("out", [1, BS], F32, isOutput=True)

    AT = mybir.AluOpType
    AF = mybir.ActivationFunctionType
    AX = mybir.AxisListType

    with tile.TileContext(nc) as tc:
        with (
            tc.tile_pool(name="const", bufs=1) as constp,
            tc.tile_pool(name="xt", bufs=6) as xtp,
            tc.tile_pool(name="wband", bufs=3) as wbp,
            tc.tile_pool(name="gt", bufs=1) as gtp,
            tc.tile_pool(name="mlp", bufs=1) as mlpp,
            tc.tile_pool(name="scratch", bufs=2) as scrp,
            tc.tile_pool(name="small", bufs=1) as smallp,
            tc.tile_pool(name="psg", bufs=2, space="PSUM") as psgp,
            tc.tile_pool(name="psh1", bufs=1, space="PSUM") as psh1p,
            tc.tile_pool(name="psh2", bufs=2, space="PSUM") as psh2p,
            tc.tile_pool(name="dram", bufs=1, space="DRAM") as dramp,
        ):
            # ---------------- constants ----------------
            # ind/wpt/gbpt gate the first wband build: put them FIRST on
            # the same (sync) ring as x so they aren't starved behind the
            # queued 1MB x streams.  Bulk weights go on the ACT ring.
            ind_sb = constp.tile([128, SUPER_SUBS * GBLK], F32)
            nc.sync.dma_start(ind_sb[:], ind_d[:])
            wpt = constp.tile([128, NSUB], F32)
            nc.sync.dma_start(wpt[:], wpt_d[:])
            gbpt = constp.tile([128, GT_TILES], F32)
            nc.sync.dma_start(gbpt[:], gbpt_d[:])
            w1t = constp.tile([128, GT_TILES * H1], BF16)
            nc.scalar.dma_start(w1t[:], w1t_d[:])
            w2t = constp.tile([128, 4 * H2], BF16)
            nc.scalar.dma_start(w2t[:], w2t_d[:])
            g1pt = constp.tile([128, 4], F32)
            nc.scalar.dma_start(g1pt[:], g1pt_d[:])
            be1pt = constp.tile([128, 4], F32)
            nc.scalar.dma_start(be1pt[:], be1pt_d[:])
            g2pt = constp.tile([128, 2], F32)
            nc.scalar.dma_start(g2pt[:], g2pt_d[:])
            be2pt = constp.tile([128, 2], F32)
            nc.scalar.dma_start(be2pt[:], be2pt_d[:])
            wopt = constp.tile([128, 2], BF16)
            nc.scalar.dma_start(wopt[:], wopt_d[:])
            boutv = constp.tile([1, 1], F32)
            nc.scalar.dma_start(boutv[:], bout_d[:])
            epst = constp.tile([128, 1], F32)
            nc.vector.memset(epst[:], EPS)

            # gT accumulator [2048(G padded) x 256] bf16: 16 partition-tiles
            # side by side.  Groups 2000..2047 are never written -> zero.
            gt = gtp.tile([128, GT_TILES * BS], BF16)
            nc.vector.memset(gt[64:128, 15 * BS:16 * BS], 0.0)

            # layer-1 pre-activations accumulate here across the main loop
            h1p = psh1p.tile([128, 4 * 512], F32)   # 4 PSUM banks, cols 0:BS used

            def build_wband(t):
                # split across DVE and ACT so neither engine paces the loop
                nsub = SUPER_SUBS if t < NSUPER - 1 else 4
                wb = wbp.tile([128, SUPER_SUBS * GBLK], BF16, tag="wband")
                for s in range(nsub):
                    c = t * SUPER_SUBS + s
                    if s % 2 == 0:
                        nc.vector.tensor_scalar_mul(
                            wb[:, s * GBLK:(s + 1) * GBLK],
                            ind_sb[:, s * GBLK:(s + 1) * GBLK],
                            wpt[:, c:c + 1],
                        )
                    else:
                        nc.scalar.activation(
                            wb[:, s * GBLK:(s + 1) * GBLK],
                            ind_sb[:, s * GBLK:(s + 1) * GBLK],
                            AF.Copy,
                            scale=wpt[:, c:c + 1],
                        )
                return wb

            def l1_matmul(k):
                # h1p[:, m] += W1T[k-block].T @ gt_k   (4 banks, 16-step accum)
                for m in range(4):
                    nc.tensor.matmul(
                        h1p[:, m * 512:m * 512 + BS],
                        w1t[:, k * H1 + m * 128:k * H1 + (m + 1) * 128],
                        gt[:, k * BS:(k + 1) * BS],
                        start=(k == 0), stop=(k == GT_TILES - 1))

            # ---------------- segment-sum main loop ----------------
            wbs = {0: build_wband(0)}
            for t in range(NSUPER):
                nsub = SUPER_SUBS if t < NSUPER - 1 else 4
                ng = GBLK if t < NSUPER - 1 else 16
                if t + 1 < NSUPER:
                    wbs[t + 1] = build_wband(t + 1)
                wb = wbs.pop(t)
                xt = xtp.tile([128, SUPER_SUBS * BS], BF16, tag="xt")
                nc.sync.dma_start(xt[:, :nsub * BS],
                                  x_d[:, t * SUPER_SUBS * BS:
                                      (t * SUPER_SUBS + nsub) * BS])
                psg = psgp.tile([64, 512], F32, tag="psg")
                for s in range(nsub):
                    nc.tensor.matmul(psg[:, 0:BS], wb[:, s * GBLK:(s + 1) * GBLK],
                                     xt[:, s * BS:(s + 1) * BS],
                                     start=(s == 0), stop=(s == nsub - 1))
                # gt[64t : 64t+ng, :] = relu(psg + gene_b)
                j, po = t // 2, 64 * (t % 2)
                nc.vector.tensor_scalar(
                    gt[po:po + ng, j * BS:(j + 1) * BS],
                    psg[0:ng, 0:BS],
                    gbpt[po:po + ng, j:j + 1],
                    0.0,
                    op0=AT.add,
                    op1=AT.max,
                )
                # interleave layer-1 accumulation one pair behind
                if t >= 3 and t % 2 == 1:
                    l1_matmul((t - 3) // 2)
            l1_matmul(15)

            # ---------------- BN1 stats + AllReduce ----------------
            # DVE sum and ACT square-sum both read PSUM; stagger them one
            # bank apart (same-bank DVE+ACT access serializes on TRN2).
            stats1 = smallp.tile([128, 8], F32)

            def stat1_sum(m):
                nc.vector.reduce_sum(stats1[:, m:m + 1],
                                     h1p[:, m * 512:m * 512 + BS], axis=AX.X)

            def stat1_sq(m):
                sq = scrp.tile([128, BS], F32, tag="sq")
                nc.scalar.activation(sq[:], h1p[:, m * 512:m * 512 + BS],
                                     AF.Square,
                                     accum_out=stats1[:, 4 + m:5 + m])

            stat1_sum(0)
            stat1_sum(1); stat1_sq(0)
            stat1_sum(2); stat1_sq(1)
            stat1_sum(3); stat1_sq(2)
            stat1_sq(3)

            bn1_in = dramp.tile([128, 8], F32)
            bn1_out = dramp.tile([128, 8], F32)
            nc.sync.dma_start(bn1_in[:], stats1[:])
            nc.gpsimd.collective_compute(
                "AllReduce", AT.add,
                replica_groups=[list(range(NCORES))],
                ins=[bn1_in.opt()], outs=[bn1_out.opt()])
            statsr1 = smallp.tile([128, 8], F32)
            nc.sync.dma_start(statsr1[:], bn1_out[:])

            mu1 = smallp.tile([128, 4], F32)
            nc.vector.tensor_scalar_mul(mu1[:], statsr1[:, 0:4], 1.0 / B)
            var1 = smallp.tile([128, 4], F32)
            nc.vector.tensor_tensor(var1[:], mu1[:], mu1[:], op=AT.mult)
            ex21 = smallp.tile([128, 4], F32)
            nc.vector.tensor_scalar_mul(ex21[:], statsr1[:, 4:8], 1.0 / B)
            nc.vector.tensor_tensor(var1[:], ex21[:], var1[:], op=AT.subtract)
            std1 = smallp.tile([128, 4], F32)
            nc.scalar.activation(std1[:], var1[:], AF.Sqrt, bias=epst[:])
            rstd1 = smallp.tile([128, 4], F32)
            nc.vector.reciprocal(rstd1[:], std1[:])
            scl1 = smallp.tile([128, 4], F32)
            nc.vector.tensor_tensor(scl1[:], g1pt[:], rstd1[:], op=AT.mult)
            shf1 = smallp.tile([128, 4], F32)
            nc.vector.tensor_tensor(shf1[:], mu1[:], scl1[:], op=AT.mult)
            nc.vector.tensor_tensor(shf1[:], be1pt[:], shf1[:], op=AT.subtract)

            h1 = mlpp.tile([128, 4 * BS], BF16)
            for m in range(4):
                nc.scalar.activation(
                    h1[:, m * BS:(m + 1) * BS], h1p[:, m * 512:m * 512 + BS],
                    AF.Relu, bias=shf1[:, m:m + 1], scale=scl1[:, m:m + 1])

            # ---------------- MLP layer 2 + BN2 ----------------
            stats2 = smallp.tile([128, 4], F32)
            ph2s = []
            for m in range(2):
                ph2 = psh2p.tile([128, 512], F32, tag="ph2")
                ph2s.append(ph2)
                for k in range(4):
                    nc.tensor.matmul(
                        ph2[:, 0:BS],
                        w2t[:, k * H2 + m * 128:k * H2 + (m + 1) * 128],
                        h1[:, k * BS:(k + 1) * BS],
                        start=(k == 0), stop=(k == 3))
                nc.vector.reduce_sum(stats2[:, m:m + 1], ph2[:, 0:BS], axis=AX.X)
            for m in range(2):
                sq2 = scrp.tile([128, BS], F32, tag="sq")
                nc.scalar.activation(sq2[:], ph2s[m][:, 0:BS], AF.Square,
                                     accum_out=stats2[:, 2 + m:3 + m])

            bn2_in = dramp.tile([128, 4], F32)
            bn2_out = dramp.tile([128, 4], F32)
            nc.sync.dma_start(bn2_in[:], stats2[:])
            nc.gpsimd.collective_compute(
                "AllReduce", AT.add,
                replica_groups=[list(range(NCORES))],
                ins=[bn2_in.opt()], outs=[bn2_out.opt()])
            statsr2 = smallp.tile([128, 4], F32)
            nc.sync.dma_start(statsr2[:], bn2_out[:])

            mu2 = smallp.tile([128, 2], F32)
            nc.vector.tensor_scalar_mul(mu2[:], statsr2[:, 0:2], 1.0 / B)
            var2 = smallp.tile([128, 2], F32)
            nc.vector.tensor_tensor(var2[:], mu2[:], mu2[:], op=AT.mult)
            ex22 = smallp.tile([128, 2], F32)
            nc.vector.tensor_scalar_mul(ex22[:], statsr2[:, 2:4], 1.0 / B)
            nc.vector.tensor_tensor(var2[:], ex22[:], var2[:], op=AT.subtract)
            std2 = smallp.tile([128, 2], F32)
            nc.scalar.activation(std2[:], var2[:], AF.Sqrt, bias=epst[:])
            rstd2 = smallp.tile([128, 2], F32)
            nc.vector.reciprocal(rstd2[:], std2[:])
            scl2 = smallp.tile([128, 2], F32)
            nc.vector.tensor_tensor(scl2[:], g2pt[:], rstd2[:], op=AT.mult)
            shf2 = smallp.tile([128, 2], F32)
            nc.vector.tensor_tensor(shf2[:], mu2[:], scl2[:], op=AT.mult)
            nc.vector.tensor_tensor(shf2[:], be2pt[:], shf2[:], op=AT.subtract)

            h2 = mlpp.tile([128, 2 * BS], BF16)
            for m in range(2):
                nc.scalar.activation(
                    h2[:, m * BS:(m + 1) * BS], ph2s[m][:, 0:BS],
                    AF.Relu, bias=shf2[:, m:m + 1], scale=scl2[:, m:m + 1])

            # ---------------- output head ----------------
            pso = psh2p.tile([128, 512], F32, tag="ph2")
            for k in range(2):
                nc.tensor.matmul(pso[0:1, 0:BS], wopt[:, k:k + 1],
                                 h2[:, k * BS:(k + 1) * BS],
                                 start=(k == 0), stop=(k == 1))
            outsb = smallp.tile([1, BS], F32)
            nc.scalar.activation(outsb[:], pso[0:1, 0:BS], AF.Identity,
                                 bias=boutv[0:1, 0:1])
            nc.sync.dma_start(out_d[:], outsb[:])

    nc.compile()
    return nc


def _pack_pt(v, ncols):
    """[N] -> [128, ncols] with element (p, c) = v[128c + p], zero padded."""
    full = np.zeros(128 * ncols, np.float32)
    full[:v.shape[0]] = v
    return np.ascontiguousarray(full.reshape(ncols, 128).T)


_GRAPH = None


def _prepare_in_maps(x, seg, w_flat, gene_b, W1, b1, gamma1, beta1, W2, b2,
                     gamma2, beta2, Wout, bout):
    x = np.asarray(x, np.float32)
    seg = np.asarray(seg)
    exp_seg = np.repeat(np.arange(G, dtype=np.int64), _SIZES)
    assert np.array_equal(seg.astype(np.int64), exp_seg), "unexpected seg layout"

    # x: bf16 cast, pad to FP, then per-core transpose to the
    # [128, NSUB*BS] streaming layout: xr[p, c*BS+b] = x[b, 128c+p]
    xb = np.zeros((B, FP), ml_dtypes.bfloat16)
    xb[:, :F] = x.astype(ml_dtypes.bfloat16)
    xr = np.ascontiguousarray(
        xb.view(np.uint16).reshape(NCORES, BS, NSUB, 128).transpose(0, 3, 2, 1)
    ).reshape(NCORES, 128, NSUB * BS).view(ml_dtypes.bfloat16)

    ind = (exp_seg[:SUPER_SUBS * 128, None] == np.arange(GBLK)[None, :])
    ind = np.ascontiguousarray(
        ind.astype(np.float32).reshape(SUPER_SUBS, 128, GBLK)
        .transpose(1, 0, 2).reshape(128, SUPER_SUBS * GBLK))
    wpt = _pack_pt(np.asarray(w_flat, np.float32), NSUB)
    gbpt = _pack_pt(np.asarray(gene_b, np.float32), GT_TILES)
    w1t_full = np.zeros((GT_TILES * 128, H1), np.float32)
    w1t_full[:G] = np.asarray(W1, np.float32).T
    w1t = np.ascontiguousarray(
        w1t_full.reshape(GT_TILES, 128, H1).transpose(1, 0, 2)
        .reshape(128, GT_TILES * H1)).astype(ml_dtypes.bfloat16)
    w2t = np.ascontiguousarray(
        np.asarray(W2, np.float32).T.reshape(4, 128, H2).transpose(1, 0, 2)
        .reshape(128, 4 * H2)).astype(ml_dtypes.bfloat16)
    g1pt = _pack_pt(np.asarray(gamma1, np.float32), 4)
    be1pt = _pack_pt(np.asarray(beta1, np.float32), 4)
    g2pt = _pack_pt(np.asarray(gamma2, np.float32), 2)
    be2pt = _pack_pt(np.asarray(beta2, np.float32), 2)
    wopt = _pack_pt(np.asarray(Wout, np.float32).reshape(-1), 2).astype(
        ml_dtypes.bfloat16)
    boutv = np.asarray(bout, np.float32).reshape(1, 1)

    consts = dict(ind=ind, wpt=wpt, gbpt=gbpt, w1t=w1t, g1pt=g1pt,
                  be1pt=be1pt, w2t=w2t, g2pt=g2pt, be2pt=be2pt,
                  wopt=wopt, boutv=boutv)
    return [dict(consts, x=np.ascontiguousarray(xr[i]))
            for i in range(NCORES)]


def _graph():
    global _GRAPH
    if _GRAPH is None:
        _GRAPH = _build_graph()
    return _GRAPH


def _gather(res):
    out = np.concatenate([np.asarray(r["out"]).reshape(-1)
                          for r in res.results])
    return out.reshape(B, 1).astype(np.float32)


def kernel(**inputs):
    in_maps = _prepare_in_maps(**inputs)
    res = run_bass_kernel_spmd(_graph(), in_maps, list(range(NCORES)))
    return _gather(res)


# revision 19
# speedup vs baseline: 1.0476x; 1.0476x over previous
"""Trainium2 Bass kernel for the GeneGroupModel two-layer problem.

Model: g = relu(segment_sum(x * w_flat, seg) + gene_b)
       h1 = relu(BN(g @ W1.T + b1));  h2 = relu(BN(h1 @ W2.T + b2))
       out = h2 @ Wout.T + bout            (BN uses full-batch statistics)

Strategy (8 NeuronCores, data-parallel over the batch):
 - batch B=2048 sharded 8 x 256 rows.
 - x is transposed + bf16-cast on the host into a [128, 469*256] layout
   (partition p, column c*256+b  =  x[b, 128c+p]) so the device streams
   it with large fully-contiguous DMAs (~1MB per super-chunk) at HBM
   line rate.  No DMA-transpose.
 - The segment structure repeats exactly every 120 features (sizes
   16/24/32/48) -> 1920 features == 64 groups.  The segment-sum becomes
   a band matmul: per 128-feature chunk, psg[64, 256] += Wband.T @ xT
   where Wband = diag(w_chunk) @ IND and IND is ONE constant [1920, 64]
   0/1 block shared by all super-chunks.  Wband is built on-chip.
 - The MLP layer-1 matmul is interleaved into the main loop (each
   completed 128-group tile of g immediately feeds 4 accumulating
   matmuls into 4 persistent PSUM banks), so only BN + layer-2 + head
   remain as a tail.  Weights and activations use bf16 (f32 PSUM
   accumulation); BN batch statistics are summed across cores with two
   tiny AllReduce collectives.
 - b1/b2 are omitted: BN subtracts the batch mean, so a constant bias
   added before BN cancels exactly.
"""

import numpy as np
import ml_dtypes

import concourse.bass as bass
import concourse.bacc as bacc
import concourse.mybir as mybir
from concourse import tile
from concourse.bass_utils import run_bass_kernel_spmd

F32 = mybir.dt.float32
BF16 = mybir.dt.bfloat16

B, F, G = 2048, 60000, 2000
H1, H2 = 512, 256
EPS = 1e-5
NCORES = 8
BS = B // NCORES            # 256 batch rows per core
NSUB = 469                  # ceil(F/128); F padded to FP
FP = NSUB * 128             # 60032
SUPER_SUBS = 15             # 15 x 128 = 1920 features per super-chunk
NSUPER = 32                 # 31 full + 1 tail (4 subchunks, 16 groups)
GBLK = 64                   # groups per full super-chunk
GT_TILES = 16               # partition tiles of gT (G padded to 2048)

_SIZES = np.tile(np.array([16, 24, 32, 48], np.int64), 500)


def _build_graph():
    nc = bacc.Bacc("TRN2", target_bir_lowering=False, debug=False,
                   num_devices=NCORES)
    x_d = nc.declare_dram_parameter("x", [128, NSUB * BS], BF16, isOutput=False)
    ind_d = nc.declare_dram_parameter("ind", [128, SUPER_SUBS * GBLK], BF16, isOutput=False)
    wpt_d = nc.declare_dram_parameter("wpt", [128, NSUB], F32, isOutput=False)
    gbpt_d = nc.declare_dram_parameter("gbpt", [128, GT_TILES], F32, isOutput=False)
    w1t_d = nc.declare_dram_parameter("w1t", [128, GT_TILES * H1], BF16, isOutput=False)
    g1pt_d = nc.declare_dram_parameter("g1pt", [128, 4], F32, isOutput=False)
    be1pt_d = nc.declare_dram_parameter("be1pt", [128, 4], F32, isOutput=False)
    w2t_d = nc.declare_dram_parameter("w2t", [128, 4 * H2], BF16, isOutput=False)
    g2pt_d = nc.declare_dram_parameter("g2pt", [128, 2], F32, isOutput=False)
    be2pt_d = nc.declare_dram_parameter("be2pt", [128, 2], F32, isOutput=False)
    wopt_d = nc.declare_dram_parameter("wopt", [128, 2], BF16, isOutput=False)
    bout_d = nc.declare_dram_parameter("boutv", [1, 1], F32, isOutput=False)
    out_d = nc.declare_dram_parameter("out", [1, BS], F32, isOutput=True)

    AT = mybir.AluOpType
    AF = mybir.ActivationFunctionType
    AX = mybir.AxisListType

    with tile.TileContext(nc) as tc:
        with (
            tc.tile_pool(name="const", bufs=1) as constp,
            tc.tile_pool(name="xt", bufs=6) as xtp,
            tc.tile_pool(name="wband", bufs=3) as wbp,
            tc.tile_pool(name="gt", bufs=1) as gtp,
            tc.tile_pool(name="mlp", bufs=1) as mlpp,
            tc.tile_pool(name="scratch", bufs=2) as scrp,
            tc.tile_pool(name="small", bufs=1) as smallp,
            tc.tile_pool(name="psg", bufs=2, space="PSUM") as psgp,
            tc.tile_pool(name="psh1", bufs=1, space="PSUM") as psh1p,
            tc.tile_pool(name="psh2", bufs=2, space="PSUM") as psh2p,
            tc.tile_pool(name="dram", bufs=1, space="DRAM") as dramp,
        ):
            # ---------------- constants ----------------
            # ind/wpt/gbpt gate the first wband build: put them FIRST on
            # the same (sync) ring as x so they aren't starved behind the
            # queued 1MB x streams.  The bulk weights (w1t etc, ACT ring)
            # are issued after wband0's ACT ops so their transfers don't
            # delay ind/wpt at the SDMA round-robin level.
            ind_sb = constp.tile([128, SUPER_SUBS * GBLK], BF16)
            nc.sync.dma_start(ind_sb[:], ind_d[:])
            wpt = constp.tile([128, NSUB], F32)
            nc.sync.dma_start(wpt[:], wpt_d[:])
            gbpt = constp.tile([128, GT_TILES], F32)
            nc.sync.dma_start(gbpt[:], gbpt_d[:])
            epst = constp.tile([128, 1], F32)
            nc.vector.memset(epst[:], EPS)

            # bulk constants on the GpSimd (SWDGE) ring: keeps both HWDGE
            # rings clear for ind/wpt/x, and the ACT queue free for wband
            # builds.  w1t first (needed from t=5).
            w1t = constp.tile([128, GT_TILES * H1], BF16)
            nc.gpsimd.dma_start(w1t[:], w1t_d[:])
            w2t = constp.tile([128, 4 * H2], BF16)
            nc.gpsimd.dma_start(w2t[:], w2t_d[:])
            g1pt = constp.tile([128, 4], F32)
            nc.gpsimd.dma_start(g1pt[:], g1pt_d[:])
            be1pt = constp.tile([128, 4], F32)
            nc.gpsimd.dma_start(be1pt[:], be1pt_d[:])
            g2pt = constp.tile([128, 2], F32)
            nc.gpsimd.dma_start(g2pt[:], g2pt_d[:])
            be2pt = constp.tile([128, 2], F32)
            nc.gpsimd.dma_start(be2pt[:], be2pt_d[:])
            wopt = constp.tile([128, 2], BF16)
            nc.gpsimd.dma_start(wopt[:], wopt_d[:])
            boutv = constp.tile([1, 1], F32)
            nc.gpsimd.dma_start(boutv[:], bout_d[:])

            # warm-up AllReduce: absorbs collective-path cold cost and
            # inter-core launch skew on the idle GpSimd queue while the
            # main loop streams x.  Result unused.
            warm_in = dramp.tile([128, 1], F32)
            warm_out = dramp.tile([128, 1], F32)
            nc.gpsimd.collective_compute(
                "AllReduce", AT.add,
                replica_groups=[list(range(NCORES))],
                ins=[warm_in.opt()], outs=[warm_out.opt()])

            # gT accumulator [2048(G padded) x 256] bf16: 16 partition-tiles
            # side by side.  Groups 2000..2047 are never written -> zero.
            gt = gtp.tile([128, GT_TILES * BS], BF16)
            nc.vector.memset(gt[64:128, 15 * BS:16 * BS], 0.0)

            # layer-1 pre-activations accumulate here across the main loop
            h1p = psh1p.tile([128, 4 * 512], F32)   # 4 PSUM banks, cols 0:BS used

            def build_wband(t):
                # split across DVE (2/3) and ACT (1/3) so neither engine
                # paces the DMA-bound loop
                nsub = SUPER_SUBS if t < NSUPER - 1 else 4
                wb = wbp.tile([128, SUPER_SUBS * GBLK], BF16, tag="wband")
                for s in range(nsub):
                    c = t * SUPER_SUBS + s
                    if s % 3 != 2:
                        nc.vector.tensor_scalar_mul(
                            wb[:, s * GBLK:(s + 1) * GBLK],
                            ind_sb[:, s * GBLK:(s + 1) * GBLK],
                            wpt[:, c:c + 1],
                        )
                    else:
                        nc.scalar.activation(
                            wb[:, s * GBLK:(s + 1) * GBLK],
                            ind_sb[:, s * GBLK:(s + 1) * GBLK],
                            AF.Copy,
                            scale=wpt[:, c:c + 1],
                        )
                return wb

            def l1_matmul(k):
                # h1p[:, m] += W1T[k-block].T @ gt_k   (4 banks, 16-step accum)
                for m in range(4):
                    nc.tensor.matmul(
                        h1p[:, m * 512:m * 512 + BS],
                        w1t[:, k * H1 + m * 128:k * H1 + (m + 1) * 128],
                        gt[:, k * BS:(k + 1) * BS],
                        start=(k == 0), stop=(k == GT_TILES - 1))

            # ---------------- segment-sum main loop ----------------
            wbs = {0: build_wband(0)}
            # bulk constants: issued here so the ACT queue triggers them
            # only after wband0's ops, keeping the startup window clear
            w1t = constp.tile([128, GT_TILES * H1], BF16)
            nc.scalar.dma_start(w1t[:], w1t_d[:])
            w2t = constp.tile([128, 4 * H2], BF16)
            nc.scalar.dma_start(w2t[:], w2t_d[:])
            g1pt = constp.tile([128, 4], F32)
            nc.scalar.dma_start(g1pt[:], g1pt_d[:])
            be1pt = constp.tile([128, 4], F32)
            nc.scalar.dma_start(be1pt[:], be1pt_d[:])
            g2pt = constp.tile([128, 2], F32)
            nc.scalar.dma_start(g2pt[:], g2pt_d[:])
            be2pt = constp.tile([128, 2], F32)
            nc.scalar.dma_start(be2pt[:], be2pt_d[:])
            wopt = constp.tile([128, 2], BF16)
            nc.scalar.dma_start(wopt[:], wopt_d[:])
            boutv = constp.tile([1, 1], F32)
            nc.scalar.dma_start(boutv[:], bout_d[:])
            for t in range(NSUPER):
                nsub = SUPER_SUBS if t < NSUPER - 1 else 4
                ng = GBLK if t < NSUPER - 1 else 16
                if t + 1 < NSUPER:
                    wbs[t + 1] = build_wband(t + 1)
                wb = wbs.pop(t)
                xt = xtp.tile([128, SUPER_SUBS * BS], BF16, tag="xt")
                nc.sync.dma_start(xt[:, :nsub * BS],
                                  x_d[:, t * SUPER_SUBS * BS:
                                      (t * SUPER_SUBS + nsub) * BS])
                psg = psgp.tile([64, 512], F32, tag="psg")
                for s in range(nsub):
                    nc.tensor.matmul(psg[:, 0:BS], wb[:, s * GBLK:(s + 1) * GBLK],
                                     xt[:, s * BS:(s + 1) * BS],
                                     start=(s == 0), stop=(s == nsub - 1))
                # gt[64t : 64t+ng, :] = relu(psg + gene_b)
                j, po = t // 2, 64 * (t % 2)
                nc.vector.tensor_scalar(
                    gt[po:po + ng, j * BS:(j + 1) * BS],
                    psg[0:ng, 0:BS],
                    gbpt[po:po + ng, j:j + 1],
                    0.0,
                    op0=AT.add,
                    op1=AT.max,
                )
                # interleave layer-1 accumulation two pairs behind (w1t
                # arrives mid-stream, so give it slack)
                if t >= 5 and t % 2 == 1:
                    l1_matmul((t - 5) // 2)
            l1_matmul(14)
            l1_matmul(15)

            # ---------------- BN1 stats + AllReduce ----------------
            # DVE sums and ACT square-sums run concurrently: separate
            # output tiles (same-tile writes serialize across engines) and
            # staggered PSUM banks (same-bank DVE+ACT access serializes).
            stats1 = smallp.tile([128, 4], F32)
            stats1b = smallp.tile([128, 4], F32)

            def stat1_sum(m):
                nc.vector.reduce_sum(stats1[:, m:m + 1],
                                     h1p[:, m * 512:m * 512 + BS], axis=AX.X)

            def stat1_sq(m):
                sq = scrp.tile([128, BS], F32, tag="sq")
                nc.scalar.activation(sq[:], h1p[:, m * 512:m * 512 + BS],
                                     AF.Square,
                                     accum_out=stats1b[:, m:m + 1])

            stat1_sum(0)
            stat1_sum(1); stat1_sq(0)
            stat1_sum(2); stat1_sq(1)
            stat1_sum(3); stat1_sq(2)
            stat1_sq(3)

            bn1_in = dramp.tile([128, 8], F32)
            bn1_out = dramp.tile([128, 8], F32)
            nc.sync.dma_start(bn1_in[:, 0:4], stats1[:])
            nc.sync.dma_start(bn1_in[:, 4:8], stats1b[:])
            nc.gpsimd.collective_compute(
                "AllReduce", AT.add,
                replica_groups=[list(range(NCORES))],
                ins=[bn1_in.opt()], outs=[bn1_out.opt()])
            statsr1 = smallp.tile([128, 8], F32)
            nc.sync.dma_start(statsr1[:], bn1_out[:])

            mu1 = smallp.tile([128, 4], F32)
            nc.vector.tensor_scalar_mul(mu1[:], statsr1[:, 0:4], 1.0 / B)
            var1 = smallp.tile([128, 4], F32)
            nc.vector.tensor_tensor(var1[:], mu1[:], mu1[:], op=AT.mult)
            ex21 = smallp.tile([128, 4], F32)
            nc.vector.tensor_scalar_mul(ex21[:], statsr1[:, 4:8], 1.0 / B)
            nc.vector.tensor_tensor(var1[:], ex21[:], var1[:], op=AT.subtract)
            std1 = smallp.tile([128, 4], F32)
            nc.scalar.activation(std1[:], var1[:], AF.Sqrt, bias=epst[:])
            rstd1 = smallp.tile([128, 4], F32)
            nc.vector.reciprocal(rstd1[:], std1[:])
            scl1 = smallp.tile([128, 4], F32)
            nc.vector.tensor_tensor(scl1[:], g1pt[:], rstd1[:], op=AT.mult)
            shf1 = smallp.tile([128, 4], F32)
            nc.vector.tensor_tensor(shf1[:], mu1[:], scl1[:], op=AT.mult)
            nc.vector.tensor_tensor(shf1[:], be1pt[:], shf1[:], op=AT.subtract)

            h1 = mlpp.tile([128, 4 * BS], BF16)
            for m in range(4):
                nc.scalar.activation(
                    h1[:, m * BS:(m + 1) * BS], h1p[:, m * 512:m * 512 + BS],
                    AF.Relu, bias=shf1[:, m:m + 1], scale=scl1[:, m:m + 1])

            # ---------------- MLP layer 2 + BN2 ----------------
            stats2 = smallp.tile([128, 2], F32)
            stats2b = smallp.tile([128, 2], F32)
            ph2s = []
            for m in range(2):
                ph2 = psh2p.tile([128, 512], F32, tag="ph2")
                ph2s.append(ph2)
                for k in range(4):
                    nc.tensor.matmul(
                        ph2[:, 0:BS],
                        w2t[:, k * H2 + m * 128:k * H2 + (m + 1) * 128],
                        h1[:, k * BS:(k + 1) * BS],
                        start=(k == 0), stop=(k == 3))
                nc.vector.reduce_sum(stats2[:, m:m + 1], ph2[:, 0:BS], axis=AX.X)
            for m in range(2):
                sq2 = scrp.tile([128, BS], F32, tag="sq")
                nc.scalar.activation(sq2[:], ph2s[m][:, 0:BS], AF.Square,
                                     accum_out=stats2b[:, m:m + 1])

            bn2_in = dramp.tile([128, 4], F32)
            bn2_out = dramp.tile([128, 4], F32)
            nc.sync.dma_start(bn2_in[:, 0:2], stats2[:])
            nc.sync.dma_start(bn2_in[:, 2:4], stats2b[:])
            nc.gpsimd.collective_compute(
                "AllReduce", AT.add,
                replica_groups=[list(range(NCORES))],
                ins=[bn2_in.opt()], outs=[bn2_out.opt()])
            statsr2 = smallp.tile([128, 4], F32)
            nc.sync.dma_start(statsr2[:], bn2_out[:])

            mu2 = smallp.tile([128, 2], F32)
            nc.vector.tensor_scalar_mul(mu2[:], statsr2[:, 0:2], 1.0 / B)
            var2 = smallp.tile([128, 2], F32)
            nc.vector.tensor_tensor(var2[:], mu2[:], mu2[:], op=AT.mult)
            ex22 = smallp.tile([128, 2], F32)
            nc.vector.tensor_scalar_mul(ex22[:], statsr2[:, 2:4], 1.0 / B)
            nc.vector.tensor_tensor(var2[:], ex22[:], var2[:], op=AT.subtract)
            std2 = smallp.tile([128, 2], F32)
            nc.scalar.activation(std2[:], var2[:], AF.Sqrt, bias=epst[:])
            rstd2 = smallp.tile([128, 2], F32)
            nc.vector.reciprocal(rstd2[:], std2[:])
            scl2 = smallp.tile([128, 2], F32)
            nc.vector.tensor_tensor(scl2[:], g2pt[:], rstd2[:], op=AT.mult)
            shf2 = smallp.tile([128, 2], F32)
            nc.vector.tensor_tensor(shf2[:], mu2[:], scl2[:], op=AT.mult)
            nc.vector.tensor_tensor(shf2[:], be2pt[:], shf2[:], op=AT.subtract)

            h2 = mlpp.tile([128, 2 * BS], BF16)
            for m in range(2):
                nc.scalar.activation(
                    h2[:, m * BS:(m + 1) * BS], ph2s[m][:, 0:BS],
                    AF.Relu, bias=shf2[:, m:m + 1], scale=scl2[:, m:m + 1])

            # ---------------- output head ----------------
            pso = psh2p.tile([128, 512], F32, tag="ph2")
            for k in range(2):
                nc.tensor.matmul(pso[0:1, 0:BS], wopt[:, k:k + 1],
                                 h2[:, k * BS:(k + 1) * BS],
                                 start=(k == 0), stop=(k == 1))
            outsb = smallp.tile([1, BS], F32)
            nc.scalar.activation(outsb[:], pso[0:1, 0:BS], AF.Identity,
                                 bias=boutv[0:1, 0:1])
            nc.sync.dma_start(out_d[:], outsb[:])

    nc.compile()
    return nc


def _pack_pt(v, ncols):
    """[N] -> [128, ncols] with element (p, c) = v[128c + p], zero padded."""
    full = np.zeros(128 * ncols, np.float32)
    full[:v.shape[0]] = v
    return np.ascontiguousarray(full.reshape(ncols, 128).T)


_GRAPH = None


def _prepare_in_maps(x, seg, w_flat, gene_b, W1, b1, gamma1, beta1, W2, b2,
                     gamma2, beta2, Wout, bout):
    x = np.asarray(x, np.float32)
    seg = np.asarray(seg)
    exp_seg = np.repeat(np.arange(G, dtype=np.int64), _SIZES)
    assert np.array_equal(seg.astype(np.int64), exp_seg), "unexpected seg layout"

    # x: bf16 cast, pad to FP, then per-core transpose to the
    # [128, NSUB*BS] streaming layout: xr[p, c*BS+b] = x[b, 128c+p]
    xb = np.zeros((B, FP), ml_dtypes.bfloat16)
    xb[:, :F] = x.astype(ml_dtypes.bfloat16)
    xr = np.ascontiguousarray(
        xb.view(np.uint16).reshape(NCORES, BS, NSUB, 128).transpose(0, 3, 2, 1)
    ).reshape(NCORES, 128, NSUB * BS).view(ml_dtypes.bfloat16)

    ind = (exp_seg[:SUPER_SUBS * 128, None] == np.arange(GBLK)[None, :])
    ind = np.ascontiguousarray(
        ind.astype(ml_dtypes.bfloat16).reshape(SUPER_SUBS, 128, GBLK)
        .transpose(1, 0, 2).reshape(128, SUPER_SUBS * GBLK))
    wpt = _pack_pt(np.asarray(w_flat, np.float32), NSUB)
    gbpt = _pack_pt(np.asarray(gene_b, np.float32), GT_TILES)
    w1t_full = np.zeros((GT_TILES * 128, H1), np.float32)
    w1t_full[:G] = np.asarray(W1, np.float32).T
    w1t = np.ascontiguousarray(
        w1t_full.reshape(GT_TILES, 128, H1).transpose(1, 0, 2)
        .reshape(128, GT_TILES * H1)).astype(ml_dtypes.bfloat16)
    w2t = np.ascontiguousarray(
        np.asarray(W2, np.float32).T.reshape(4, 128, H2).transpose(1, 0, 2)
        .reshape(128, 4 * H2)).astype(ml_dtypes.bfloat16)
    g1pt = _pack_pt(np.asarray(gamma1, np.float32), 4)
    be1pt = _pack_pt(np.asarray(beta1, np.float32), 4)
    g2pt = _pack_pt(np.asarray(gamma2, np.float32), 2)
    be2pt = _pack_pt(np.asarray(beta2, np.float32), 2)
    wopt = _pack_pt(np.asarray(Wout, np.float32).reshape(-1), 2).astype(
        ml_dtypes.bfloat16)
    boutv = np.asarray(bout, np.float32).reshape(1, 1)

    consts = dict(ind=ind, wpt=wpt, gbpt=gbpt, w1t=w1t, g1pt=g1pt,
                  be1pt=be1pt, w2t=w2t, g2pt=g2pt, be2pt=be2pt,
                  wopt=wopt, boutv=boutv)
    return [dict(consts, x=np.ascontiguousarray(xr[i]))
            for i in range(NCORES)]


def _graph():
    global _GRAPH
    if _GRAPH is None:
        _GRAPH = _build_graph()
    return _GRAPH


def _gather(res):
    out = np.concatenate([np.asarray(r["out"]).reshape(-1)
                          for r in res.results])
    return out.reshape(B, 1).astype(np.float32)


def kernel(**inputs):
    in_maps = _prepare_in_maps(**inputs)
    res = run_bass_kernel_spmd(_graph(), in_maps, list(range(NCORES)))
    return _gather(res)
